# revision 1
# baseline (speedup 1.0000x reference)
"""Trainium2 Bass kernel: negative-Jacobian-determinant penalty loss.

reference semantics:
    y = identity_grid + y_pred            # [B, D, H, W, 3]
    J = np.gradient-style central/one-sided diffs of y along (D, H, W)
    det = det3x3(J) per voxel; loss = mean(min(det, 0)^2)

Math used here:
  * gradient(identity_grid) == 1 exactly everywhere (incl. edges), so
    J = I + G with G[j][c] = grad_j(y_pred[c]).
  * one-sided edge diffs == central diffs over a linearly-extrapolated
    1-voxel pad, so the host pads and the device does only central diffs.
  * central diff = 0.5*(f[+1]-f[-1]); we compute raw diffs D = 2*G and
    det(I+G) = det(2I + D)/8, folding the /8 (squared: /64) into the
    final host-side scale.

Device layout (per core, shard = one (batch, D-quarter)):
  host-padded, host-transposed shard x[3, 42, 226, 195]  (c, d, w, h);
  Hp is odd so the +-1-element H-diff streams stay 4-byte aligned and
  keep the DVE fp16 2x perf mode.
  partitions = W (two 128-row chunks), free dim = (d, h).
  a = D-diffs (free-dim shift +-Hp), c = H-diffs (free-dim shift +-1),
  b = W-diffs via TensorE shift-matrix matmul (PSUM) + ScalarE exit
  (which also folds in the +2 diagonal offset via a per-partition bias).
  negdet = -det(2I+D) via DVE fp16 products (two of the six cross-term
  muls run on GPSIMD to share load); relu via DVE max(.,0);
  Square+accumulate via ScalarE activation accum_out; per-chunk
  W-validity masks applied in a final tiny TensorE matmul.
"""

import math
import os
import sys
from contextlib import ExitStack
from dataclasses import dataclass

import numpy as np

for _p in ("/root/.axon_site/_ro/trn_rl_repo", "/opt/trn_rl_repo"):
    if os.path.isdir(_p) and _p not in sys.path:
        sys.path.append(_p)

import concourse.bass as bass  # noqa: E402
import concourse.mybir as mybir  # noqa: E402
import concourse.tile as tile  # noqa: E402
from concourse import bacc  # noqa: E402
from concourse import bass_utils  # noqa: E402

F32 = mybir.dt.float32
F16 = mybir.dt.float16


@dataclass(frozen=True)
class Cfg:
    Dsh: int = 42   # shard D planes incl 1-plane halo each side
    Wp: int = 226   # padded W (1 halo each side)
    Hp: int = 195   # padded H (2 low / 1 high pad; odd so +-1 shifts are 4B-aligned)
    kD: int = 12    # output D planes per group
    P: int = 128    # partition rows per W chunk
    dtype: str = "f16"  # on-chip compute dtype: "f16" or "f32"
    mm_sub: int = 2  # D planes per PE matmul sub-chunk (free <= 512)
    use_xs: bool = False  # GPSIMD shifted copy -> aligned H-diffs (moot with odd Hp)
    gp_muls: int = 2      # how many det-stage muls to offload to GPSIMD (0/2/4)
    xbufs: int = 2        # pool bufs: x tiles
    fbufs: int = 2        # pool bufs: field tiles (a/c/b)
    mbufs: int = 2        # pool bufs: det-stage scratch tiles
    act_relu: bool = True  # relu on ScalarE (frees DVE) instead of DVE max
    act_c1: bool = False   # +2 offset on c1 via ScalarE instead of DVE TS
    pe_adiff: bool = False  # D-diffs via TensorE + wide ACT exits (crashes HW; sim-only)

    @property
    def dt(self):
        return F16 if self.dtype == "f16" else F32

    @property
    def npdt(self):
        return np.float16 if self.dtype == "f16" else np.float32

    @property
    def w_chunks(self):
        """[(cw0, valid_lo, valid_hi)] local partition rows, inclusive."""
        chunks = []
        lo = 1  # first valid global w row
        last = self.Wp - 2
        while lo <= last:
            cw0 = min(lo - 1, self.Wp - self.P)
            hi = min(cw0 + self.P - 2, last)
            chunks.append((cw0, lo - cw0, hi - cw0))
            lo = hi + 1
        return chunks

    @property
    def d_groups(self):
        """[(d0, n_out)] group reads planes d0..d0+n_out+1, outputs d0+1..d0+n_out."""
        groups = []
        d0 = 0
        last = self.Dsh - 2
        while d0 < last:
            n = min(self.kD, last - d0)
            groups.append((d0, n))
            d0 += n
        return groups


def _consts(cfg: Cfg):
    """Host-side constant tensors: shift matrix + bias/mask columns."""
    P = cfg.P
    sc = np.zeros((P, P), dtype=np.float32)
    for m in range(P):
        if m + 1 < P:
            sc[m + 1, m] = 1.0
        if m - 1 >= 0:
            sc[m - 1, m] = -1.0
    chunks = cfg.w_chunks
    bm = np.zeros((P, 2 + len(chunks)), dtype=np.float32)
    bm[:, 1] = 2.0
    for ci, (_, lo, hi) in enumerate(chunks):
        bm[lo : hi + 1, 2 + ci] = 1.0
    ids = np.concatenate([np.eye(P, dtype=np.float32), -np.eye(P, dtype=np.float32)], axis=1)
    return {"sc": sc.astype(cfg.npdt), "bm": bm, "ids": ids.astype(cfg.npdt)}


def build_nc(cfg: Cfg):
    nc = bacc.Bacc("TRN2", target_bir_lowering=False, debug=False)
    P, Hp, kD = cfg.P, cfg.Hp, cfg.kD
    dt = cfg.dt
    chunks = cfg.w_chunks
    groups = cfg.d_groups
    n_slots = len(chunks) * len(groups)

    x_d = nc.dram_tensor("x", [3, cfg.Dsh, cfg.Wp, Hp], F32, kind="ExternalInput").ap()
    sc_d = nc.dram_tensor("sc", [P, P], dt, kind="ExternalInput").ap()
    bm_d = nc.dram_tensor("bm", [P, 2 + len(chunks)], F32, kind="ExternalInput").ap()
    ids_d = nc.dram_tensor("ids", [P, 2 * P], dt, kind="ExternalInput").ap() if cfg.pe_adiff else None
    out_d = nc.dram_tensor("out", [1, 1], F32, kind="ExternalOutput").ap()

    cast = dt != F32

    with tile.TileContext(nc) as tc, ExitStack() as ctx:
        cpool = ctx.enter_context(tc.tile_pool(name="consts", bufs=1))
        xpool = ctx.enter_context(tc.tile_pool(name="x", bufs=cfg.xbufs))
        fpool = ctx.enter_context(tc.tile_pool(name="fields", bufs=cfg.fbufs))
        mpool = ctx.enter_context(tc.tile_pool(name="mags", bufs=cfg.mbufs))
        apool = ctx.enter_context(tc.tile_pool(name="acc", bufs=1))
        pp = ctx.enter_context(
            tc.tile_pool(name="psum", bufs=2 if cfg.pe_adiff else 4, space="PSUM")
        )
        pa = (
            ctx.enter_context(tc.tile_pool(name="apsum", bufs=2, space="PSUM"))
            if cfg.pe_adiff
            else None
        )
        fp = ctx.enter_context(tc.tile_pool(name="fpsum", bufs=1, space="PSUM"))

        sc_sb = cpool.tile([P, P], dt)
        nc.sync.dma_start(sc_sb[:], sc_d)
        bm_sb = cpool.tile([P, 2 + len(chunks)], F32)
        nc.sync.dma_start(bm_sb[:], bm_d)
        zvec = bm_sb[:, 0:1]
        if cfg.pe_adiff:
            ids_sb = cpool.tile([P, 2 * P], dt)
            nc.sync.dma_start(ids_sb[:], ids_d)
            id_pos, id_neg = ids_sb[:, 0:P], ids_sb[:, P : 2 * P]

        acc = apool.tile([P, n_slots], F32)

        for ci, (cw0, _, _) in enumerate(chunks):
            for gi, (d0, nD) in enumerate(groups):
                KD2 = nD + 2
                Fx = KD2 * Hp
                Ff = nD * Hp
                xt = []
                for ch in range(3):
                    t = xpool.tile([P, Fx], dt, tag=f"x{ch}", name=f"x{ch}")
                    src = x_d[ch, d0 : d0 + KD2, cw0 : cw0 + P, :].rearrange(
                        "d w h -> w d h"
                    )
                    dst = t[:].rearrange("p (d h) -> p d h", d=KD2)
                    (nc.gpsimd if cast else nc.sync).dma_start(dst, src)
                    xt.append(t)

                pe_a = cfg.pe_adiff and nD % 4 == 0
                a = []
                c = []
                for ch in range(3):
                    at = fpool.tile([P, Ff], dt, tag=f"a{ch}", name=f"a{ch}")
                    if pe_a:
                        # D-diff on TensorE: +I @ X[d+1] - I @ X[d-1], PSUM-
                        # accumulated, exited 4 planes at a time on ScalarE
                        # (2-bank PSUM tile, bank-aligned halves) with the
                        # +2 diagonal offset folded into channel 0's bias.
                        for j in range(nD // 4):
                            ap4 = pa.tile([P, 1024], F32, tag="ap4", name="ap4")
                            for half in range(2):
                                p0 = 4 * j + 2 * half
                                dstp = ap4[:, half * 512 : half * 512 + 2 * Hp]
                                nc.tensor.matmul(
                                    dstp,
                                    id_pos,
                                    xt[ch][:, (p0 + 2) * Hp : (p0 + 4) * Hp],
                                    start=True,
                                    stop=False,
                                )
                                nc.tensor.matmul(
                                    dstp,
                                    id_neg,
                                    xt[ch][:, p0 * Hp : (p0 + 2) * Hp],
                                    start=False,
                                    stop=True,
                                )
                            src3 = ap4[:].rearrange("p (k x) -> p k x", k=2)[
                                :, :, 0 : 2 * Hp
                            ]
                            dsta = a_dst = at[:, j * 4 * Hp : (j + 1) * 4 * Hp]
                            if ch == 0:
                                nc.scalar.activation(
                                    dsta,
                                    src3,
                                    mybir.ActivationFunctionType.Identity,
                                    bias=bm_sb[:, 1:2],
                                    scale=1.0,
                                )
                            else:
                                nc.scalar.copy(dsta, src3)
                    else:
                        nc.vector.tensor_sub(
                            at[:], xt[ch][:, 2 * Hp : 2 * Hp + Ff], xt[ch][:, 0:Ff]
                        )
                    a.append(at)
                    if cfg.use_xs:
                        # GPSIMD copy of X shifted by one element so both
                        # H-diff streams start 4B-aligned (DVE 2x mode).
                        xs = fpool.tile([P, Ff + 2], dt, tag=f"xs{ch}", name=f"xs{ch}")
                        nc.gpsimd.tensor_copy(
                            xs[:], xt[ch][:, Hp - 1 : Hp - 1 + Ff + 2]
                        )
                        ct = fpool.tile([P, Ff], dt, tag=f"c{ch}", name=f"c{ch}")
                        nc.vector.tensor_sub(ct[:], xs[:, 2 : 2 + Ff], xs[:, 0:Ff])
                    else:
                        ct = fpool.tile([P, Ff], dt, tag=f"c{ch}", name=f"c{ch}")
                        nc.vector.tensor_sub(
                            ct[:],
                            xt[ch][:, Hp + 1 : Hp + 1 + Ff],
                            xt[ch][:, Hp - 1 : Hp - 1 + Ff],
                        )
                    c.append(ct)
                # diagonal +2 offsets (rows are D,H,W derivs; channels 0,1,2)
                if not pe_a:
                    nc.vector.tensor_scalar_add(a[0][:], a[0][:], 2.0)
                if cfg.act_c1:
                    nc.scalar.activation(
                        c[1][:], c[1][:], mybir.ActivationFunctionType.Identity,
                        bias=bm_sb[:, 1:2], scale=1.0,
                    )
                else:
                    nc.vector.tensor_scalar_add(c[1][:], c[1][:], 2.0)

                b = [fpool.tile([P, Ff], dt, tag=f"b{ch}", name=f"b{ch}") for ch in range(3)]
                sub = cfg.mm_sub
                for p0 in range(0, nD, sub):
                    pn = min(sub, nD - p0)
                    for ch in range(3):
                        bp = pp.tile([P, pn * Hp], F32, tag="bpsum", name="bpsum")
                        rhs = xt[ch][:, (1 + p0) * Hp : (1 + p0 + pn) * Hp]
                        nc.tensor.matmul(bp[:], sc_sb[:], rhs, start=True, stop=True)
                        dst = b[ch][:, p0 * Hp : (p0 + pn) * Hp]
                        if ch == 2:
                            nc.scalar.activation(
                                dst,
                                bp[:],
                                mybir.ActivationFunctionType.Identity,
                                bias=bm_sb[:, 1:2],
                                scale=1.0,
                            )
                        else:
                            nc.scalar.copy(dst, bp[:])

                def tt(tag, fn, u, v):
                    o = mpool.tile([P, Ff], dt, tag=tag, name=tag)
                    fn(o[:], u[:], v[:])
                    return o

                mul, sub_ = nc.vector.tensor_mul, nc.vector.tensor_sub
                add_ = nc.vector.tensor_add
                gmul = nc.gpsimd.tensor_mul
                mul3a = gmul if cfg.gp_muls >= 3 else mul
                mul3b = gmul if cfg.gp_muls >= 4 else mul
                mul5 = gmul if cfg.gp_muls >= 2 else mul
                # negdet = a . (b x c)  ==  -(a . (c x b))
                m1 = tt("m1", mul, c[2], b[1])
                m2 = tt("m2", mul, c[1], b[2])
                x0 = tt("x0", sub_, m1, m2)
                tg3 = "m3" if cfg.gp_muls >= 3 else "m1"
                tg4 = "m4" if cfg.gp_muls >= 4 else "m2"
                m3 = tt(tg3, mul3a, c[0], b[2])
                m4 = tt(tg4, mul3b, c[2], b[0])
                x1 = tt("x1", sub_, m3, m4)
                m5 = tt("m5", mul5, c[1], b[0])
                m6 = tt("m6", mul5, c[0], b[1])
                x2 = tt("x2", sub_, m5, m6)
                t0 = tt("m1", mul, a[0], x0)
                t1 = tt("m2", mul, a[1], x1)
                t2 = tt("x0", mul, a[2], x2)
                s = tt("x1", add_, t0, t1)
                s2 = tt("x2", add_, s, t2)
                r = mpool.tile([P, Ff], dt, tag="r", name="r")
                if cfg.act_relu:
                    nc.scalar.activation(
                        r[:], s2[:], mybir.ActivationFunctionType.Relu,
                        bias=zvec, scale=1.0,
                    )
                else:
                    nc.vector.tensor_scalar_max(r[:], s2[:], 0.0)

                junk = mpool.tile([P, nD * (Hp - 3)], dt, tag="junk", name="junk")
                rw = r[:].rearrange("p (d h) -> p d h", d=nD)[:, :, 2 : Hp - 1]
                jw = junk[:].rearrange("p (d h) -> p d h", d=nD)
                slot = ci * len(groups) + gi
                nc.scalar.activation(
                    jw,
                    rw,
                    mybir.ActivationFunctionType.Square,
                    bias=zvec,
                    scale=1.0,
                    accum_out=acc[:, slot : slot + 1],
                )

        fin = fp.tile([1, 1], F32)
        for ci in range(len(chunks)):
            accC = apool.tile([P, 1], F32, tag=f"accC{ci}", name=f"accC{ci}")
            nc.vector.tensor_reduce(
                accC[:],
                acc[:, ci * len(groups) : (ci + 1) * len(groups)],
                axis=mybir.AxisListType.X,
                op=mybir.AluOpType.add,
            )
            nc.tensor.matmul(
                fin[:],
                bm_sb[:, 2 + ci : 3 + ci],
                accC[:],
                start=(ci == 0),
                stop=(ci == len(chunks) - 1),
            )
        outs = apool.tile([1, 1], F32, tag="outs", name="outs")
        nc.scalar.copy(outs[:], fin[:])
        nc.sync.dma_start(out_d, outs[:])

    nc.compile()
    return nc


# ----------------------------------------------------------------------------
# host-side data prep
# ----------------------------------------------------------------------------

def _pad_extrap(x, axis, n_lo, n_hi):
    """Pad with linear extrapolation of the edge (1 real pad plane), then
    replicate it for any extra (never-consumed, finiteness-only) planes."""
    def take(i):
        sl = [slice(None)] * x.ndim
        sl[axis] = slice(i, i + 1) if i >= 0 else slice(i, None if i == -1 else i + 1)
        return x[tuple(sl)]

    lo = 2.0 * take(0) - take(1)
    hi = 2.0 * take(-1) - take(-2)
    parts = [lo] * n_lo + [x] + [hi] * n_hi
    return np.concatenate(parts, axis=axis)


def prepare_shards(y_pred: np.ndarray, cfg: Cfg):
    """[B, 3, D, H, W] fp32 -> list of 8 shards [3, Dsh, Wp, Hp] (c,d,w,h)."""
    B, C, D, H, W = y_pred.shape
    x = np.asarray(y_pred, dtype=np.float32)
    x = _pad_extrap(x, 2, 1, 1)          # D -> D+2
    x = _pad_extrap(x, 3, 2, 1)          # H -> H+3 (odd Hp)
    x = _pad_extrap(x, 4, 1, 1)          # W -> W+2
    x = np.ascontiguousarray(x.transpose(0, 1, 2, 4, 3))  # [B, 3, D+2, W+2, H+4]
    nq = 8 // B
    dq = D // nq
    shards = []
    for b in range(B):
        for q in range(nq):
            shards.append(np.ascontiguousarray(x[b, :, dq * q : dq * q + dq + 2]))
    return shards


def shard_ref_sum(xs: np.ndarray, cfg: Cfg) -> float:
    """Numpy mirror of the device computation for one shard (for testing)."""
    x = xs.astype(np.float64)
    _, Dsh, Wp, Hp = x.shape
    dd, ww, hh = slice(1, Dsh - 1), slice(1, Wp - 1), slice(2, Hp - 1)
    a = x[:, 2:, ww, hh] - x[:, : Dsh - 2, ww, hh]
    c = x[:, dd, ww, 3:Hp] - x[:, dd, ww, 1 : Hp - 2]
    b = x[:, dd, 2:, hh] - x[:, dd, : Wp - 2, hh]
    a[0] += 2.0
    c[1] += 2.0
    b[2] += 2.0
    det = (
        a[0] * (c[1] * b[2] - c[2] * b[1])
        - a[1] * (c[0] * b[2] - c[2] * b[0])
        + a[2] * (c[0] * b[1] - c[1] * b[0])
    )
    neg = np.maximum(-det, 0.0)
    return float(np.sum(neg * neg))


# ----------------------------------------------------------------------------
# entry point
# ----------------------------------------------------------------------------

_CACHE: dict = {}


def _get_nc(cfg: Cfg):
    if cfg not in _CACHE:
        _CACHE[cfg] = build_nc(cfg)
    return _CACHE[cfg]


def run_shards(shards, cfg: Cfg, trace=False):
    nc = _get_nc(cfg)
    consts = _consts(cfg)
    if not cfg.pe_adiff:
        consts = {k: v for k, v in consts.items() if k != "ids"}
    in_maps = [{"x": s, **consts} for s in shards]
    res = bass_utils.run_bass_kernel_spmd(
        nc, in_maps, core_ids=list(range(len(shards))), trace=trace
    )
    sums = [float(r["out"][0, 0]) for r in res.results]
    return sums, res


def kernel(y_pred: np.ndarray) -> np.ndarray:
    dty = os.environ.get("DETK_DTYPE", "f16")
    B, C, D, H, W = y_pred.shape
    nq = 8 // B
    cfg = Cfg(
        Dsh=D // nq + 2,
        Wp=W + 2,
        Hp=H + 3,
        dtype=dty,
        kD=12 if dty == "f16" else 4,
    )
    shards = prepare_shards(y_pred, cfg)
    sums, _ = run_shards(shards, cfg)
    total = math.fsum(sums)
    mean = total / 64.0 / float(B * D * H * W)
    return np.array(mean, dtype=np.float32)


if __name__ == "__main__":
    np.random.seed(0)
    yp = np.random.randn(2, 3, 160, 192, 224).astype(np.float32)
    print(kernel(yp))

